# revision 44
# baseline (speedup 1.0000x reference)
"""Distributed Trainium2 kernel for nn_AltBlock (dense transformer block).

Sharding: 2-way batch x 4-way tensor parallel. Core c owns batch c//4 and
head group g=c%4 (heads 4g..4g+3) plus MLP hidden slice [1024g, 1024(g+1)).
All activations stay in transposed [feature_part, token_free] layout, so no
device-side activation transposes are needed (x arrives pre-transposed from
host). LayerNorm gains are folded into the weights on host; LN stats are
computed with f32r ones-matmuls and rank-1 broadcasts on the PE.

Attention: S^T = kn^T qn per head with tokens streaming 512-wide; softmax is
exp(S - bound) * exp(alibi + mask) where exp(alibi+mask) is precomputed on
host in bf16 (turns the alibi add into a cheap elementwise multiply); row
sums are folded into the V-matmul via a ones column appended to V.

proj and fc2 produce partial sums that are AllReduced (bf16, add) within
each 4-core batch group, token-halved so the collectives overlap MLP
compute. Output is written transposed in bf16 and fixed up on host.
"""

import math
import numpy as np
from contextlib import ExitStack

B, N, C, H = 2, 1024, 1024, 16
D = C // H          # 64
HID = 4 * C         # 4096
NCORES = 8
GROUP = 4
EPS = 1e-5

_CACHE = {}


def _build_nc():
    import concourse.bass as bass
    import concourse.tile as tile
    from concourse import bacc, mybir

    f32 = mybir.dt.float32
    f32r = mybir.dt.float32r
    bf16 = mybir.dt.bfloat16
    AF = mybir.ActivationFunctionType
    OP = mybir.AluOpType

    nc = bacc.Bacc(None, target_bir_lowering=False)

    xT_in = nc.dram_tensor("xT", [8, 128, N], f32r, kind="ExternalInput")
    eal_in = nc.dram_tensor("ealibi", [4, 8, 128, N], bf16, kind="ExternalInput")
    wqkv_in = nc.dram_tensor("wqkv", [8, 128, 768], bf16, kind="ExternalInput")
    wproj_in = nc.dram_tensor("wproj", [2, 128, 1024], bf16, kind="ExternalInput")
    w1_in = nc.dram_tensor("w1", [8, 128, 1024], bf16, kind="ExternalInput")
    w2_in = nc.dram_tensor("w2", [8, 128, 1024], bf16, kind="ExternalInput")
    bqkv_in = nc.dram_tensor("bqkvT", [128, 6], f32, kind="ExternalInput")
    bproj_in = nc.dram_tensor("bprojT", [128, 8], f32, kind="ExternalInput")
    b1_in = nc.dram_tensor("b1T", [128, 8], f32, kind="ExternalInput")
    b2_in = nc.dram_tensor("b2T", [128, 8], f32, kind="ExternalInput")
    scalesq_in = nc.dram_tensor("scalesq", [4, 1], f32, kind="ExternalInput")
    nbT_in = nc.dram_tensor("nbT", [128, 4], f32, kind="ExternalInput")
    cf_in = nc.dram_tensor("cf", [128, 900], f32r, kind="ExternalInput")
    cb_in = nc.dram_tensor("cb", [128, 160], bf16, kind="ExternalInput")
    out_ext = nc.dram_tensor("out", [8, 128, N], bf16, kind="ExternalOutput")

    with ExitStack() as stack:
        stack.enter_context(nc.allow_low_precision(reason="f32r views of f32"))
        tc = stack.enter_context(tile.TileContext(nc))
        pP = stack.enter_context(tc.tile_pool(name="pP", bufs=1))
        pdram = stack.enter_context(tc.tile_pool(name="pdram", bufs=1, space="DRAM"))

        # ---- constants / persistents ----
        cf = pP.tile([128, 900], f32r, name="cf")
        nc.sync.dma_start(cf, cf_in[:])
        ones128 = cf[:, 0:1]           # [128,1] all-ones (stats lhsT)
        sel2T = cf[0:2, 1:129]         # [2,128]: p<64 <- row0, p>=64 <- row1
        selR = cf[0:2, 129:257]        # [2,128]: broadcast row 0 to all p
        selM = cf[0:2, 257:385]       # [2,128]: broadcast row 1 to all p
        # selA[f]: [8,128] broadcast rows 2f/2f+1 to partition halves
        selA = [cf[0:8, 385 + 128 * f:385 + 128 * (f + 1)] for f in range(4)]

        cb = pP.tile([128, 160], bf16, name="cb")
        nc.sync.dma_start(cb, cb_in[:])
        ident = cb[:, 0:128]
        # sel8[:, f, 2f+i]: indicator of partition half i -> accumulating
        # per-head sum-of-squares rows for fb pair f
        sel8 = cb[:, 128:160].rearrange("p (a b) -> p a b", a=4)

        bqkv_sb = pP.tile([128, 6], f32, name="bqkv_sb")
        nc.sync.dma_start(bqkv_sb, bqkv_in[:])
        bproj_sb = pP.tile([128, 8], f32, name="bproj_sb")
        nc.sync.dma_start(bproj_sb, bproj_in[:])
        b1_sb = pP.tile([128, 8], f32, name="b1_sb")
        nc.sync.dma_start(b1_sb, b1_in[:])
        b2_sb = pP.tile([128, 8], f32, name="b2_sb")
        nc.sync.dma_start(b2_sb, b2_in[:])
        scalesq_sb = pP.tile([4, 1], f32, name="scalesq_sb")
        nc.sync.dma_start(scalesq_sb, scalesq_in[:])
        nbT_sb = pP.tile([128, 4], f32, name="nbT_sb")
        nc.sync.dma_start(nbT_sb, nbT_in[:])
        eps_sb = pP.tile([128, 1], f32, name="eps_sb")
        nc.vector.memset(eps_sb, EPS)

        xT = pP.tile([128, 8, N], f32r, name="xT")
        for cc in range(8):
            nc.sync.dma_start(xT[:, cc, :], xT_in[cc])

        # weights persist (prefetched early; only 56KB/partition total)
        wqkv_sb = pP.tile([128, 8, 768], bf16, name="wqkv_sb")
        for cc in range(8):
            nc.sync.dma_start(wqkv_sb[:, cc, :], wqkv_in[cc])
        wproj_sb = pP.tile([128, 2, 1024], bf16, name="wproj_sb")
        for rc in range(2):
            nc.sync.dma_start(wproj_sb[:, rc, :], wproj_in[rc])
        w1_sb = pP.tile([128, 8, 1024], bf16, name="w1_sb")
        w2_sb = pP.tile([128, 8, 1024], bf16, name="w2_sb")
        for cc in range(8):
            nc.sync.dma_start(w1_sb[:, cc, :], w1_in[cc])
        for cc in range(8):
            nc.sync.dma_start(w2_sb[:, cc, :], w2_in[cc])

        OT_n = pP.tile([128, 2, N], bf16, name="OT_n")

        # ---------------- layernorm helper (transposed layout) -------------
        def ln_half(src, j, hdst, st_mu, st_sq, psBC, ptmp, ptmpV, ptmpG):
            """LN over channel dim for token half j; src [128,8,N] f32r,
            hdst [128,8,N] bf16."""
            jsl = slice(j * 512, (j + 1) * 512)
            for cc in range(8):
                nc.tensor.matmul(st_mu[:, j, :], lhsT=ones128,
                                 rhs=src[:, cc, jsl],
                                 start=(cc == 0), stop=(cc == 7),
                                 skip_group_check=True)
            for cc in range(8):
                xq = ptmp.tile([128, 512], f32r, name="xq", tag="xq", bufs=2)
                nc.scalar.activation(out=xq, in_=src[:, cc, jsl], func=AF.Square)
                nc.tensor.matmul(st_sq[:, j, :], lhsT=ones128, rhs=xq,
                                 start=(cc == 0), stop=(cc == 7),
                                 skip_group_check=True)
            tmu = ptmp.tile([1, 512], f32, name="tmu", tag="tmu", bufs=1)
            tms = ptmp.tile([1, 512], f32, name="tms", tag="tms", bufs=1)
            tvv = ptmp.tile([1, 512], f32, name="tvv", tag="tvv", bufs=1)
            trs = ptmp.tile([1, 512], f32r, name="trs", tag="trs", bufs=1)
            tmr = ptmp.tile([1, 512], f32r, name="tmr", tag="tmr", bufs=1)
            nc.vector.tensor_scalar(out=tmu, in0=st_mu[:, j, :],
                                    scalar1=1.0 / C, scalar2=None, op0=OP.mult)
            nc.scalar.activation(out=tms, in_=tmu, func=AF.Square)
            nc.vector.tensor_scalar(out=tvv, in0=st_sq[:, j, :],
                                    scalar1=1.0 / C, scalar2=None, op0=OP.mult)
            nc.vector.tensor_sub(tvv, tvv, tms)
            nc.scalar.activation(out=tms, in_=tvv, func=AF.Sqrt,
                                 bias=eps_sb[0:1, 0:1])
            nc.vector.reciprocal(trs, tms)
            nc.vector.tensor_mul(tmr, trs, tmu)
            bc = psBC.tile([128, 2, 512], f32, name="bc", tag="bc")
            for s, row in enumerate((trs, tmr)):
                nc.tensor.matmul(bc[:, s, :], lhsT=selR[0:1, :], rhs=row,
                                 start=True, stop=True)
            bcs = ptmp.tile([128, 2, 512], f32, name="bcs", tag="bcs", bufs=2)
            nc.vector.tensor_copy(bcs, bc)
            for cc in range(8):
                eng = nc.vector if cc % 2 == 0 else nc.gpsimd
                tp = (ptmpV if cc % 2 == 0 else ptmpG).tile(
                    [128, 512], f32, name="apl", tag="apl", bufs=2)
                eng.tensor_mul(tp, src[:, cc, jsl], bcs[:, 0, :])
                eng.tensor_sub(hdst[:, cc, jsl], tp, bcs[:, 1, :])

        # ================= Phase A: LN1 + qkv(q,k) + norms =================
        ptmpA = stack.enter_context(tc.tile_pool(name="ptmpA", bufs=2))
        ptmpV = stack.enter_context(tc.tile_pool(name="ptmpV", bufs=2))
        ptmpG = stack.enter_context(tc.tile_pool(name="ptmpG", bufs=2))
        pAB = stack.enter_context(tc.tile_pool(name="pAB", bufs=1))

        qn_t = pAB.tile([128, 4, N], bf16, name="qn_t")
        v_kd = pAB.tile([128, 8, 4, 65], bf16, name="v_kd")

        stackA = ExitStack()
        pA = stackA.enter_context(tc.tile_pool(name="pA", bufs=1))
        qkvT = pA.tile([128, 6, N], bf16, name="qkvT")
        hT = pA.tile([128, 8, N], bf16, name="hT")

        with tc.tile_pool(name="psLN", bufs=1, space="PSUM") as psLN:
            st_mu = psLN.tile([1, 2, 512], f32, name="st_mu")
            st_sq = psLN.tile([1, 2, 512], f32, name="st_sq")
            with tc.tile_pool(name="psBC", bufs=2, space="PSUM") as psBC:
                for j in range(2):
                    ln_half(xT, j, hT, st_mu, st_sq, psBC, ptmpA, ptmpV, ptmpG)

        with tc.tile_pool(name="psQ", bufs=2, space="PSUM") as psQ:
            # q,k supers
            for fb in range(4):
                ps = psQ.tile([128, 2, 512], f32, name="qps", tag="mm")
                for cc in range(8):
                    for j in range(2):
                        nc.tensor.matmul(
                            ps[:, j, :],
                            lhsT=wqkv_sb[:, cc, fb * 128:(fb + 1) * 128],
                            rhs=hT[:, cc, j * 512:(j + 1) * 512],
                            start=(cc == 0), stop=(cc == 7),
                            skip_group_check=True)
                nc.scalar.activation(out=qkvT[:, fb, :],
                                     in_=ps.rearrange("p a b -> p (a b)"),
                                     func=AF.Identity,
                                     bias=bqkv_sb[:, fb:fb + 1], scale=1.0)
            # q/k norms
            with tc.tile_pool(name="psN", bufs=1, space="PSUM") as psN:
                q2 = pA.tile([128, 4, N], bf16, name="q2")
                nc.vector.tensor_mul(q2[:, 0:2, :], qkvT[:, 0:2, :],
                                     qkvT[:, 0:2, :])
                nc.gpsimd.tensor_mul(q2[:, 2:4, :], qkvT[:, 2:4, :],
                                     qkvT[:, 2:4, :])
                ssq = psN.tile([8, 2, 512], f32, name="ssq")
                for f in range(4):
                    for j in range(2):
                        nc.tensor.matmul(
                            ssq[:, j, :], lhsT=sel8[:, f, :],
                            rhs=q2[:, f, j * 512:(j + 1) * 512],
                            start=(f == 0), stop=(f == 3),
                            skip_group_check=True)
                rn = pA.tile([8, 2, 512], f32, name="rn")
                nc.scalar.activation(out=rn, in_=ssq, func=AF.Sqrt,
                                     bias=eps_sb[0:8, 0:1])
                rnr = pA.tile([8, 2, 512], f32r, name="rnr")
                nc.vector.reciprocal(rnr, rn)
                nc.vector.tensor_scalar(out=rnr[0:4], in0=rnr[0:4],
                                        scalar1=scalesq_sb, scalar2=None,
                                        op0=OP.mult)
                for f in range(4):
                    bcn = psN.tile([128, 2, 512], f32, name="bcn", tag="bcn")
                    for j in range(2):
                        nc.tensor.matmul(bcn[:, j, :], lhsT=selA[f],
                                         rhs=rnr[:, j, :],
                                         start=True, stop=True)
                    nc.vector.tensor_mul(qn_t[:, f, :], qkvT[:, f, :],
                                         bcn.rearrange("p a b -> p (a b)"))
            # v supers
            for fv in range(2):
                ps = psQ.tile([128, 2, 512], f32, name="qps", tag="mm")
                for cc in range(8):
                    for j in range(2):
                        nc.tensor.matmul(
                            ps[:, j, :],
                            lhsT=wqkv_sb[:, cc, (4 + fv) * 128:(5 + fv) * 128],
                            rhs=hT[:, cc, j * 512:(j + 1) * 512],
                            start=(cc == 0), stop=(cc == 7),
                            skip_group_check=True)
                nc.scalar.activation(out=qkvT[:, 4 + fv, :],
                                     in_=ps.rearrange("p a b -> p (a b)"),
                                     func=AF.Identity,
                                     bias=bqkv_sb[:, 4 + fv:5 + fv], scale=1.0)
            # v transposes -> v_kd [k_part, kc, head, d|ones]
            nc.gpsimd.memset(v_kd[:, :, :, 64:65], 1.0)
            with tc.tile_pool(name="psT", bufs=2, space="PSUM") as psT:
                for fv in range(2):
                    for kc in range(8):
                        tp = psT.tile([128, 128], bf16, name="tp", tag="tp")
                        nc.tensor.transpose(
                            tp, qkvT[:, 4 + fv, kc * 128:(kc + 1) * 128], ident)
                        dst = v_kd[:, kc, 2 * fv:2 * fv + 2, 0:64]
                        nc.scalar.activation(
                            out=dst, in_=tp.rearrange("p (a b) -> p a b", a=2),
                            func=AF.Identity, bias=0.0, scale=1.0)
        stackA.close()

        # ================= Phase B: attention =================
        with tc.tile_pool(name="pB", bufs=1) as pB, \
             tc.tile_pool(name="peal", bufs=2) as peal, \
             tc.tile_pool(name="pPT", bufs=2) as pPT, \
             tc.tile_pool(name="psS", bufs=2, space="PSUM") as psS, \
             tc.tile_pool(name="psO", bufs=2, space="PSUM") as psO, \
             tc.tile_pool(name="psB", bufs=1, space="PSUM") as psB:
            for h in range(4):
                fbq, fbk, rr = h // 2, 2 + h // 2, 64 * (h % 2)
                eal_t = peal.tile([128, 8, N], bf16, name="eal", tag="eal")
                nc.sync.dma_start(eal_t, eal_in[h].rearrange("a p b -> p a b"))
                PT = pPT.tile([128, 8, N], bf16, name="PT", tag="pt")
                for kc in range(8):
                    for j in range(2):
                        S = psS.tile([128, 512], f32, name="S", tag="s")
                        nc.tensor.matmul(
                            S,
                            lhsT=qn_t[rr:rr + 64, fbk, kc * 128:(kc + 1) * 128],
                            rhs=qn_t[rr:rr + 64, fbq, j * 512:(j + 1) * 512],
                            start=True, stop=True)
                        jsl = slice(j * 512, (j + 1) * 512)
                        nc.scalar.activation(out=PT[:, kc, jsl], in_=S,
                                             func=AF.Exp,
                                             bias=nbT_sb[:, h:h + 1], scale=1.0)
                        eng = nc.vector if (kc + j) % 2 == 0 else nc.gpsimd
                        eng.tensor_mul(PT[:, kc, jsl], PT[:, kc, jsl],
                                       eal_t[:, kc, jsl])
                O = psO.tile([65, 2, 512], f32, name="O", tag="o")
                for kc in range(8):
                    for j in range(2):
                        nc.tensor.matmul(
                            O[:, j, :], lhsT=v_kd[:, kc, h, :],
                            rhs=PT[:, kc, j * 512:(j + 1) * 512],
                            start=(kc == 0), stop=(kc == 7),
                            skip_group_check=True)
                rs = pB.tile([1, 2, 512], f32r, name="rs", tag="rs", bufs=2)
                nc.vector.reciprocal(rs, O[64:65, :, :])
                bcn2 = psB.tile([128, 2, 512], f32, name="bcn2", tag="b2")
                for j in range(2):
                    nc.tensor.matmul(bcn2[:, j, :], lhsT=selR[0:1, :],
                                     rhs=rs[:, j, :], start=True, stop=True)
                bcs2 = pB.tile([64, 2, 512], bf16, name="bcs2", tag="bc2", bufs=2)
                nc.vector.tensor_copy(bcs2, bcn2[0:64])
                r2 = 64 * (h % 2)
                nc.vector.tensor_mul(
                    OT_n[r2:r2 + 64, h // 2, :],
                    O[0:64].rearrange("p a b -> p (a b)"),
                    bcs2.rearrange("p a b -> p (a b)"))

        # ============ Phase C: proj + AR1 + LN2 + MLP + AR2 ============
        ar1_d = [nc.dram_tensor(f"ar1o_{j}", [128, 4096], bf16, kind="Internal")
                 for j in range(2)]
        ar2_d = [nc.dram_tensor(f"ar2o_{j}", [128, 4096], bf16, kind="Internal")
                 for j in range(2)]
        bounce1 = [pdram.tile([128, 4096], bf16, name=f"bounce1_{j}")
                   for j in range(2)]
        bounce2 = [pdram.tile([128, 4096], bf16, name=f"bounce2_{j}")
                   for j in range(2)]
        groups = [[0, 1, 2, 3], [4, 5, 6, 7]]

        with tc.tile_pool(name="pC", bufs=1) as pC, \
             tc.tile_pool(name="psM", bufs=2, space="PSUM") as psM, \
             tc.tile_pool(name="psLN2", bufs=1, space="PSUM") as psLN2, \
             tc.tile_pool(name="psBC2", bufs=1, space="PSUM") as psBC2:
            # rotate 4 big buffers: projT/f2T, ar1_sb/ar2_sb, y2T/outT share
            def cbig(nm):
                return pC.tile([128, 8, N], bf16, name=nm, tag="cbig", bufs=4)
            projT = cbig("projT")
            ar1_sb = cbig("ar1_sb")
            y2T = cbig("y2T")
            h1T = cbig("h1T")
            f2T = cbig("f2T")
            ar2_sb = cbig("ar2_sb")
            outT = cbig("outT")
            st2_mu = psLN2.tile([1, 2, 512], f32, name="st2_mu")
            st2_sq = psLN2.tile([1, 2, 512], f32, name="st2_sq")

            # proj partials per token half, AR as soon as each half is done
            for j in range(2):
                jsl = slice(j * 512, (j + 1) * 512)
                for ob in range(8):
                    ps = psM.tile([128, 512], f32, name="pps", tag="mm")
                    for rc in range(2):
                        nc.tensor.matmul(ps, lhsT=wproj_sb[:, rc,
                                                           ob * 128:(ob + 1) * 128],
                                         rhs=OT_n[:, rc, jsl],
                                         start=(rc == 0), stop=(rc == 1))
                    if ob % 2 == 0:
                        nc.scalar.activation(out=projT[:, ob, jsl], in_=ps,
                                             func=AF.Identity,
                                             bias=bproj_sb[:, ob:ob + 1],
                                             scale=1.0)
                    else:
                        nc.vector.tensor_scalar(
                            out=projT[:, ob, jsl], in0=ps,
                            scalar1=bproj_sb[:, ob:ob + 1], scalar2=None,
                            op0=OP.add)
                nc.sync.dma_start(
                    bounce1[j].rearrange("p (a b) -> p a b", a=8),
                    projT[:, :, jsl])
                nc.gpsimd.collective_compute(
                    "AllReduce", OP.add,
                    ins=[bounce1[j].opt()], outs=[ar1_d[j][:].opt()],
                    replica_groups=groups)
                nc.sync.dma_start(ar1_sb[:, :, jsl],
                                  ar1_d[j][:].rearrange("p (a b) -> p a b", a=8))

            for j in range(2):
                jsl = slice(j * 512, (j + 1) * 512)
                # residual (in place into xT) then LN2 on the half
                for cc in range(8):
                    eng = nc.vector if cc % 2 == 0 else nc.gpsimd
                    eng.tensor_add(xT[:, cc, jsl], xT[:, cc, jsl],
                                   ar1_sb[:, cc, jsl])
                ln_half(xT, j, y2T, st2_mu, st2_sq, psBC2, ptmpA, ptmpV, ptmpG)
                # fc1 + gelu for the half
                for hb in range(8):
                    ps = psM.tile([128, 512], f32, name="m1ps", tag="mm")
                    for cc in range(8):
                        nc.tensor.matmul(
                            ps, lhsT=w1_sb[:, cc, hb * 128:(hb + 1) * 128],
                            rhs=y2T[:, cc, jsl],
                            start=(cc == 0), stop=(cc == 7))
                    nc.scalar.activation(out=h1T[:, hb, jsl], in_=ps,
                                         func=AF.Gelu,
                                         bias=b1_sb[:, hb:hb + 1], scale=1.0)
            for j in range(2):
                jsl = slice(j * 512, (j + 1) * 512)
                for ob in range(8):
                    ps = psM.tile([128, 512], f32, name="m2ps", tag="mm")
                    for hc in range(8):
                        nc.tensor.matmul(
                            ps, lhsT=w2_sb[:, hc, ob * 128:(ob + 1) * 128],
                            rhs=h1T[:, hc, jsl],
                            start=(hc == 0), stop=(hc == 7))
                    if ob % 2 == 0:
                        nc.scalar.activation(out=f2T[:, ob, jsl], in_=ps,
                                             func=AF.Identity,
                                             bias=b2_sb[:, ob:ob + 1], scale=1.0)
                    else:
                        nc.vector.tensor_scalar(
                            out=f2T[:, ob, jsl], in0=ps,
                            scalar1=b2_sb[:, ob:ob + 1], scalar2=None,
                            op0=OP.add)
                nc.sync.dma_start(
                    bounce2[j].rearrange("p (a b) -> p a b", a=8),
                    f2T[:, :, jsl])
                nc.gpsimd.collective_compute(
                    "AllReduce", OP.add,
                    ins=[bounce2[j].opt()], outs=[ar2_d[j][:].opt()],
                    replica_groups=groups)
                nc.sync.dma_start(ar2_sb[:, :, jsl],
                                  ar2_d[j][:].rearrange("p (a b) -> p a b", a=8))
            # final residual + output
            for j in range(2):
                jsl = slice(j * 512, (j + 1) * 512)
                for cc in range(8):
                    eng = nc.vector if cc % 2 == 0 else nc.gpsimd
                    eng.tensor_add(outT[:, cc, jsl], xT[:, cc, jsl],
                                   ar2_sb[:, cc, jsl])
            for cc in range(8):
                nc.sync.dma_start(out_ext[cc], outT[:, cc, :])

    nc.finalize()
    return nc


def _get_nc():
    if "nc" not in _CACHE:
        _CACHE["nc"] = _build_nc()
    return _CACHE["nc"]


def _make_in_maps(inputs):
    import ml_dtypes
    bf = ml_dtypes.bfloat16
    x = np.asarray(inputs["x"], np.float32)
    mask = np.asarray(inputs["padding_mask"]).astype(bool)
    alibi = np.asarray(inputs["alibi_bias"], np.float32)
    g1 = np.asarray(inputs["ln1_g"], np.float32)
    b1ln = np.asarray(inputs["ln1_b"], np.float32)
    g2 = np.asarray(inputs["ln2_g"], np.float32)
    b2ln = np.asarray(inputs["ln2_b"], np.float32)
    Wqkv = np.asarray(inputs["Wqkv"], np.float32)
    bqkv = np.asarray(inputs["bqkv"], np.float32)
    Wproj = np.asarray(inputs["Wproj"], np.float32)
    bproj = np.asarray(inputs["bproj"], np.float32)
    W1 = np.asarray(inputs["W1"], np.float32)
    b1 = np.asarray(inputs["b1"], np.float32)
    W2 = np.asarray(inputs["W2"], np.float32)
    b2 = np.asarray(inputs["b2"], np.float32)
    ls = np.asarray(inputs["logit_scale"], np.float32).reshape(H)
    scale = np.exp(np.minimum(ls, math.log(100.0))).astype(np.float32)

    Wqkv_eff = g1[:, None] * Wqkv
    bqkv_eff = b1ln @ Wqkv + bqkv
    W1_eff = g2[:, None] * W1
    b1_eff = b2ln @ W1 + b1

    cf = np.zeros((128, 900), np.float32)
    cf[:, 0] = 1.0
    cf[0, 1:65] = 1.0       # sel2T row0 -> p<64
    cf[1, 65:129] = 1.0     # sel2T row1 -> p>=64
    cf[0, 129:257] = 1.0    # selR
    cf[1, 257:385] = 1.0    # selM
    for f in range(4):
        c0 = 385 + 128 * f
        cf[2 * f, c0:c0 + 64] = 1.0
        cf[2 * f + 1, c0 + 64:c0 + 128] = 1.0
    cbm = np.zeros((128, 160), np.float32)
    cbm[:, 0:128] = np.eye(128, dtype=np.float32)
    for f in range(4):
        cbm[0:64, 128 + 8 * f + 2 * f] = 1.0
        cbm[64:128, 128 + 8 * f + 2 * f + 1] = 1.0

    maskadd = np.where(mask, np.float32(-1e9), np.float32(0.0))

    in_maps = []
    for c in range(NCORES):
        b, g = divmod(c, GROUP)
        heads = [4 * g + jj for jj in range(4)]
        # qkv column gather: fb order q01 q23 k01 k23 v01 v23
        cols = []
        for f in range(6):
            t = [0, 0, 1, 1, 2, 2][f]
            pair = (heads[0], heads[1]) if f % 2 == 0 else (heads[2], heads[3])
            for hh in pair:
                cols.extend(range(t * C + hh * D, t * C + hh * D + D))
        cols = np.array(cols)
        rows_proj = np.concatenate(
            [np.arange(hh * D, (hh + 1) * D) for hh in heads])

        a = alibi[b, heads].transpose(0, 2, 1) + maskadd[b][None, :, None]
        eal = np.exp(a).astype(bf).reshape(4, 8, 128, N)

        m = {
            "xT": np.ascontiguousarray(x[b].T).reshape(8, 128, N),
            "ealibi": np.ascontiguousarray(eal),
            "wqkv": np.ascontiguousarray(
                Wqkv_eff[:, cols].reshape(8, 128, 768)).astype(bf),
            "wproj": np.ascontiguousarray(
                Wproj[rows_proj, :].reshape(2, 128, 1024)).astype(bf),
            "w1": np.ascontiguousarray(
                W1_eff[:, g * 1024:(g + 1) * 1024].reshape(8, 128, 1024)
            ).astype(bf),
            "w2": np.ascontiguousarray(
                W2[g * 1024:(g + 1) * 1024, :].reshape(8, 128, 1024)).astype(bf),
            "bqkvT": np.ascontiguousarray(bqkv_eff[cols].reshape(6, 128).T),
            "bprojT": np.ascontiguousarray(bproj.reshape(8, 128).T)
            if g == 0 else np.zeros((128, 8), np.float32),
            "b1T": np.ascontiguousarray(
                b1_eff[g * 1024:(g + 1) * 1024].reshape(8, 128).T),
            "b2T": np.ascontiguousarray(b2.reshape(8, 128).T)
            if g == 0 else np.zeros((128, 8), np.float32),
            "scalesq": np.ascontiguousarray(scale[heads].reshape(4, 1)),
            "nbT": np.ascontiguousarray(
                np.tile(-scale[heads][None, :], (128, 1))),
            "cf": cf,
            "cb": cbm.astype(bf),
        }
        in_maps.append(m)
    return in_maps


def _run(inputs, trace=False):
    from concourse import bass_utils
    nc = _get_nc()
    in_maps = _make_in_maps(inputs)
    res = bass_utils.run_bass_kernel_spmd(
        nc, in_maps, core_ids=list(range(NCORES)), trace=trace)
    ys = []
    for b in range(B):
        o = np.asarray(res.results[b * GROUP]["out"]).astype(np.float32)
        ys.append(o.reshape(C, N).T)          # [cc*128+p, t] -> [t, c]
    y = np.stack(ys)                           # [B, N, C]
    return y.astype(np.float32), res


def kernel(**inputs):
    y, _ = _run(inputs, trace=False)
    return y


# revision 59
# speedup vs baseline: 1.2892x; 1.2892x over previous
"""Distributed Trainium2 kernel for nn_AltBlock (dense transformer block).

Sharding: 2-way batch x 4-way tensor parallel. Core c owns batch c//4 and
head group g=c%4 (heads 4g..4g+3) plus MLP hidden slice [1024g, 1024(g+1)).
All activations stay in transposed [feature_part, token_free] layout, so no
device-side activation transposes are needed (x arrives pre-transposed from
host). LayerNorm gains are folded into the weights on host; LN stats are
computed with f32r ones-matmuls and rank-1 broadcasts on the PE.

Attention: S^T = kn^T qn per head with tokens streaming 512-wide; softmax is
exp(S - bound) * exp(alibi + mask) where exp(alibi+mask) is precomputed on
host in bf16 (turns the alibi add into a cheap elementwise multiply); row
sums are folded into the V-matmul via a ones column appended to V.

proj and fc2 produce partial sums that are AllReduced (bf16, add) within
each 4-core batch group, token-halved so the collectives overlap MLP
compute. Output is written transposed in bf16 and fixed up on host.
"""

import math
import numpy as np
from contextlib import ExitStack

B, N, C, H = 2, 1024, 1024, 16
D = C // H          # 64
HID = 4 * C         # 4096
NCORES = 8
GROUP = 4
EPS = 1e-5

_CACHE = {}


def _build_nc():
    import concourse.bass as bass
    import concourse.tile as tile
    from concourse import bacc, mybir

    f32 = mybir.dt.float32
    f32r = mybir.dt.float32r
    bf16 = mybir.dt.bfloat16
    AF = mybir.ActivationFunctionType
    OP = mybir.AluOpType

    nc = bacc.Bacc(None, target_bir_lowering=False)

    xT_in = nc.dram_tensor("xT", [8, 128, N], f32r, kind="ExternalInput")
    eal_in = nc.dram_tensor("ealibi", [4, 8, 128, N], bf16, kind="ExternalInput")
    wqkv_in = nc.dram_tensor("wqkv", [8, 128, 768], bf16, kind="ExternalInput")
    wproj_in = nc.dram_tensor("wproj", [2, 128, 1024], bf16, kind="ExternalInput")
    w1_in = nc.dram_tensor("w1", [32, 128, 8, 128], bf16, kind="ExternalInput")
    w2_in = nc.dram_tensor("w2", [8, 128, 32, 128], bf16, kind="ExternalInput")
    xq_in = nc.dram_tensor("xq", [8, 128, 2, 128], f32r, kind="ExternalInput")
    bqkv_in = nc.dram_tensor("bqkvT", [128, 6], f32, kind="ExternalInput")
    bproj_in = nc.dram_tensor("bprojT", [128, 8], f32, kind="ExternalInput")
    b1_in = nc.dram_tensor("b1T", [128, 32], f32, kind="ExternalInput")
    b2_in = nc.dram_tensor("b2T", [128, 8], f32, kind="ExternalInput")
    scalesq_in = nc.dram_tensor("scalesq", [4, 1], f32, kind="ExternalInput")
    nbT_in = nc.dram_tensor("nbT", [128, 4], f32, kind="ExternalInput")
    cf_in = nc.dram_tensor("cf", [128, 900], f32r, kind="ExternalInput")
    cb_in = nc.dram_tensor("cb", [128, 160], bf16, kind="ExternalInput")
    out_ext = nc.dram_tensor("out", [8, 128, 256], bf16, kind="ExternalOutput")

    with ExitStack() as stack:
        stack.enter_context(nc.allow_low_precision(reason="f32r views of f32"))
        tc = stack.enter_context(tile.TileContext(nc))
        pP = stack.enter_context(tc.tile_pool(name="pP", bufs=1))
        pdram = stack.enter_context(tc.tile_pool(name="pdram", bufs=1, space="DRAM"))

        # ---- constants / persistents ----
        cf = pP.tile([128, 900], f32r, name="cf")
        nc.sync.dma_start(cf, cf_in[:])
        ones128 = cf[:, 0:1]           # [128,1] all-ones (stats lhsT)
        sel2T = cf[0:2, 1:129]         # [2,128]: p<64 <- row0, p>=64 <- row1
        selR = cf[0:2, 129:257]        # [2,128]: broadcast row 0 to all p
        selM = cf[0:2, 257:385]       # [2,128]: broadcast row 1 to all p
        # selA[f]: [8,128] broadcast rows 2f/2f+1 to partition halves
        selA = [cf[0:8, 385 + 128 * f:385 + 128 * (f + 1)] for f in range(4)]

        cb = pP.tile([128, 160], bf16, name="cb")
        nc.sync.dma_start(cb, cb_in[:])
        ident = cb[:, 0:128]
        # sel8[:, f, 2f+i]: indicator of partition half i -> accumulating
        # per-head sum-of-squares rows for fb pair f
        sel8 = cb[:, 128:160].rearrange("p (a b) -> p a b", a=4)

        bqkv_sb = pP.tile([128, 6], f32, name="bqkv_sb")
        nc.sync.dma_start(bqkv_sb, bqkv_in[:])
        bproj_sb = pP.tile([128, 8], f32, name="bproj_sb")
        nc.sync.dma_start(bproj_sb, bproj_in[:])
        b1_sb = pP.tile([128, 32], f32, name="b1_sb")
        nc.sync.dma_start(b1_sb, b1_in[:])
        b2_sb = pP.tile([128, 8], f32, name="b2_sb")
        nc.sync.dma_start(b2_sb, b2_in[:])
        scalesq_sb = pP.tile([4, 1], f32, name="scalesq_sb")
        nc.sync.dma_start(scalesq_sb, scalesq_in[:])
        nbT_sb = pP.tile([128, 4], f32, name="nbT_sb")
        nc.sync.dma_start(nbT_sb, nbT_in[:])
        eps_sb = pP.tile([128, 1], f32, name="eps_sb")
        nc.vector.memset(eps_sb, EPS)



        # weights persist (prefetched early)
        wqkv_sb = pP.tile([128, 8, 768], bf16, name="wqkv_sb")
        for cc in range(8):
            nc.sync.dma_start(wqkv_sb[:, cc, :], wqkv_in[cc])
        wproj_sb = pP.tile([128, 2, 1024], bf16, name="wproj_sb")
        for rc in range(2):
            nc.sync.dma_start(wproj_sb[:, rc, :], wproj_in[rc])
        xq_sb = pP.tile([128, 8, 2, 128], f32r, name="xq_sb")
        for cc in range(8):
            nc.sync.dma_start(xq_sb[:, cc], xq_in[cc])

        OT_n = pP.tile([128, 2, N], bf16, name="OT_n")

        # ---------------- layernorm helper (transposed layout) -------------
        def ln_half(src, j, hdst, st_mu, st_sq, psBC, ptmp, ptmpV, ptmpG):
            """LN over channel dim for token half j; src [128,8,N] f32r,
            hdst [128,8,N] bf16."""
            jsl = slice(j * 512, (j + 1) * 512)
            for cc in range(8):
                nc.tensor.matmul(st_mu[:, j, :], lhsT=ones128,
                                 rhs=src[:, cc, jsl],
                                 start=(cc == 0), stop=(cc == 7),
                                 skip_group_check=True)
            for cc in range(8):
                xq = ptmp.tile([128, 512], f32r, name="xq", tag="xq", bufs=2)
                nc.scalar.activation(out=xq, in_=src[:, cc, jsl], func=AF.Square)
                nc.tensor.matmul(st_sq[:, j, :], lhsT=ones128, rhs=xq,
                                 start=(cc == 0), stop=(cc == 7),
                                 skip_group_check=True)
            tmu = ptmp.tile([1, 512], f32, name="tmu", tag="tmu", bufs=1)
            tms = ptmp.tile([1, 512], f32, name="tms", tag="tms", bufs=1)
            tvv = ptmp.tile([1, 512], f32, name="tvv", tag="tvv", bufs=1)
            trs = ptmp.tile([1, 512], f32r, name="trs", tag="trs", bufs=1)
            tmr = ptmp.tile([1, 512], f32r, name="tmr", tag="tmr", bufs=1)
            nc.vector.tensor_scalar(out=tmu, in0=st_mu[:, j, :],
                                    scalar1=1.0 / C, scalar2=None, op0=OP.mult)
            nc.scalar.activation(out=tms, in_=tmu, func=AF.Square)
            nc.vector.tensor_scalar(out=tvv, in0=st_sq[:, j, :],
                                    scalar1=1.0 / C, scalar2=None, op0=OP.mult)
            nc.vector.tensor_sub(tvv, tvv, tms)
            nc.scalar.activation(out=tms, in_=tvv, func=AF.Sqrt,
                                 bias=eps_sb[0:1, 0:1])
            nc.vector.reciprocal(trs, tms)
            nc.vector.tensor_mul(tmr, trs, tmu)
            bc = psBC.tile([128, 2, 512], f32, name="bc", tag="bc")
            for s, row in enumerate((trs, tmr)):
                nc.tensor.matmul(bc[:, s, :], lhsT=selR[0:1, :], rhs=row,
                                 start=True, stop=True)
            bcs = ptmp.tile([128, 2, 512], f32, name="bcs", tag="bcs", bufs=2)
            nc.vector.tensor_copy(bcs, bc)
            for cc in range(8):
                eng = nc.vector if cc % 2 == 0 else nc.gpsimd
                tp = (ptmpV if cc % 2 == 0 else ptmpG).tile(
                    [128, 512], f32, name="apl", tag="apl", bufs=2)
                eng.tensor_mul(tp, src[:, cc, jsl], bcs[:, 0, :])
                eng.tensor_sub(hdst[:, cc, jsl], tp, bcs[:, 1, :])

        # ================= Phase A: LN1 + qkv(q,k) + norms =================
        ptmpA = stack.enter_context(tc.tile_pool(name="ptmpA", bufs=2))
        ptmpV = stack.enter_context(tc.tile_pool(name="ptmpV", bufs=2))
        ptmpG = stack.enter_context(tc.tile_pool(name="ptmpG", bufs=2))
        pAB = stack.enter_context(tc.tile_pool(name="pAB", bufs=1))

        qn_t = pAB.tile([128, 4, N], bf16, name="qn_t")
        v_kd = pAB.tile([128, 8, 4, 65], bf16, name="v_kd")

        stackA = ExitStack()
        pA = stackA.enter_context(tc.tile_pool(name="pA", bufs=1))
        qkvT = pA.tile([128, 6, N], bf16, name="qkvT")
        hT = pA.tile([128, 8, N], bf16, name="hT")
        xT = pA.tile([128, 8, N], f32r, name="xT")
        for cc in range(8):
            nc.sync.dma_start(xT[:, cc, :], xT_in[cc])

        with tc.tile_pool(name="psLN", bufs=1, space="PSUM") as psLN:
            st_mu = psLN.tile([1, 2, 512], f32, name="st_mu")
            st_sq = psLN.tile([1, 2, 512], f32, name="st_sq")
            with tc.tile_pool(name="psBC", bufs=2, space="PSUM") as psBC:
                for j in range(2):
                    ln_half(xT, j, hT, st_mu, st_sq, psBC, ptmpA, ptmpV, ptmpG)

        with tc.tile_pool(name="psQ", bufs=2, space="PSUM") as psQ:
            # q,k supers
            for fb in range(4):
                ps = psQ.tile([128, 2, 512], f32, name="qps", tag="mm")
                for cc in range(8):
                    for j in range(2):
                        nc.tensor.matmul(
                            ps[:, j, :],
                            lhsT=wqkv_sb[:, cc, fb * 128:(fb + 1) * 128],
                            rhs=hT[:, cc, j * 512:(j + 1) * 512],
                            start=(cc == 0), stop=(cc == 7),
                            skip_group_check=True)
                nc.scalar.activation(out=qkvT[:, fb, :],
                                     in_=ps.rearrange("p a b -> p (a b)"),
                                     func=AF.Identity,
                                     bias=bqkv_sb[:, fb:fb + 1], scale=1.0)
            # q/k norms
            with tc.tile_pool(name="psN", bufs=1, space="PSUM") as psN:
                q2 = pA.tile([128, 4, N], bf16, name="q2")
                nc.vector.tensor_mul(q2[:, 0:2, :], qkvT[:, 0:2, :],
                                     qkvT[:, 0:2, :])
                nc.gpsimd.tensor_mul(q2[:, 2:4, :], qkvT[:, 2:4, :],
                                     qkvT[:, 2:4, :])
                ssq = psN.tile([8, 2, 512], f32, name="ssq")
                for f in range(4):
                    for j in range(2):
                        nc.tensor.matmul(
                            ssq[:, j, :], lhsT=sel8[:, f, :],
                            rhs=q2[:, f, j * 512:(j + 1) * 512],
                            start=(f == 0), stop=(f == 3),
                            skip_group_check=True)
                rn = pA.tile([8, 2, 512], f32, name="rn")
                nc.scalar.activation(out=rn, in_=ssq, func=AF.Sqrt,
                                     bias=eps_sb[0:8, 0:1])
                rnr = pA.tile([8, 2, 512], f32r, name="rnr")
                nc.vector.reciprocal(rnr, rn)
                nc.vector.tensor_scalar(out=rnr[0:4], in0=rnr[0:4],
                                        scalar1=scalesq_sb, scalar2=None,
                                        op0=OP.mult)
                for f in range(4):
                    bcn = psN.tile([128, 2, 512], f32, name="bcn", tag="bcn")
                    for j in range(2):
                        nc.tensor.matmul(bcn[:, j, :], lhsT=selA[f],
                                         rhs=rnr[:, j, :],
                                         start=True, stop=True)
                    nc.vector.tensor_mul(qn_t[:, f, :], qkvT[:, f, :],
                                         bcn.rearrange("p a b -> p (a b)"))
            # v supers
            for fv in range(2):
                ps = psQ.tile([128, 2, 512], f32, name="qps", tag="mm")
                for cc in range(8):
                    for j in range(2):
                        nc.tensor.matmul(
                            ps[:, j, :],
                            lhsT=wqkv_sb[:, cc, (4 + fv) * 128:(5 + fv) * 128],
                            rhs=hT[:, cc, j * 512:(j + 1) * 512],
                            start=(cc == 0), stop=(cc == 7),
                            skip_group_check=True)
                nc.scalar.activation(out=qkvT[:, 4 + fv, :],
                                     in_=ps.rearrange("p a b -> p (a b)"),
                                     func=AF.Identity,
                                     bias=bqkv_sb[:, 4 + fv:5 + fv], scale=1.0)
            # v transposes -> v_kd [k_part, kc, head, d|ones]
            nc.gpsimd.memset(v_kd[:, :, :, 64:65], 1.0)
            with tc.tile_pool(name="psT", bufs=2, space="PSUM") as psT:
                for fv in range(2):
                    for kc in range(8):
                        tp = psT.tile([128, 128], bf16, name="tp", tag="tp")
                        nc.tensor.transpose(
                            tp, qkvT[:, 4 + fv, kc * 128:(kc + 1) * 128], ident)
                        dst = v_kd[:, kc, 2 * fv:2 * fv + 2, 0:64]
                        nc.scalar.activation(
                            out=dst, in_=tp.rearrange("p (a b) -> p a b", a=2),
                            func=AF.Identity, bias=0.0, scale=1.0)
        stackA.close()

        # ============ Phase B: attention + proj + ReduceScatter ============
        # token-halved: half j's proj partials ReduceScatter (token-quarters
        # within the half) while the other half's attention runs.
        rs_d = [nc.dram_tensor(f"rso_{j}", [128, 8, 128], bf16, kind="Internal")
                for j in range(2)]
        bounce1 = [pdram.tile([4, 128, 8, 128], bf16, name=f"bounce1_{j}")
                   for j in range(2)]
        groups = [[0, 1, 2, 3], [4, 5, 6, 7]]

        pw1 = stack.enter_context(tc.tile_pool(name="pw1", bufs=4))
        pw2 = stack.enter_context(tc.tile_pool(name="pw2", bufs=3))

        pBo = stack.enter_context(tc.tile_pool(name="pBo", bufs=1))
        projT = pBo.tile([128, 8, N], bf16, name="projT")
        rs_sb = pBo.tile([128, 8, 2, 128], bf16, name="rs_sb")

        with tc.tile_pool(name="pB", bufs=1) as pB, \
             tc.tile_pool(name="peal", bufs=3) as peal, \
             tc.tile_pool(name="pPT", bufs=3) as pPT, \
             tc.tile_pool(name="psS", bufs=4, space="PSUM") as psS, \
             tc.tile_pool(name="psO", bufs=2, space="PSUM") as psO, \
             tc.tile_pool(name="psB", bufs=2, space="PSUM") as psB:
            for j in range(2):
                jsl = slice(j * 512, (j + 1) * 512)
                for h in range(4):
                    fbq, fbk, rr = h // 2, 2 + h // 2, 64 * (h % 2)
                    eal_t = peal.tile([128, 8, 512], bf16, name="eal", tag="eal")
                    nc.sync.dma_start(
                        eal_t, eal_in[h, :, :, jsl].rearrange("a p b -> p a b"))
                    PT = pPT.tile([128, 8, 512], bf16, name="PT", tag="pt")
                    for kc in range(8):
                        S = psS.tile([128, 512], f32, name="S", tag="s")
                        nc.tensor.matmul(
                            S,
                            lhsT=qn_t[rr:rr + 64, fbk, kc * 128:(kc + 1) * 128],
                            rhs=qn_t[rr:rr + 64, fbq, jsl],
                            start=True, stop=True)
                        nc.scalar.activation(out=PT[:, kc, :], in_=S,
                                             func=AF.Exp,
                                             bias=nbT_sb[:, h:h + 1], scale=1.0)
                        eng = nc.vector if kc % 2 == 0 else nc.gpsimd
                        eng.tensor_mul(PT[:, kc, :], PT[:, kc, :],
                                       eal_t[:, kc, :])
                    O = psO.tile([65, 512], f32, name="O", tag="o")
                    for kc in range(8):
                        nc.tensor.matmul(
                            O, lhsT=v_kd[:, kc, h, :], rhs=PT[:, kc, :],
                            start=(kc == 0), stop=(kc == 7),
                            skip_group_check=True)
                    rs = pB.tile([1, 512], f32r, name="rs", tag="rs", bufs=2)
                    nc.vector.reciprocal(rs, O[64:65, :])
                    bcn2 = psB.tile([128, 512], f32, name="bcn2", tag="b2")
                    nc.tensor.matmul(bcn2, lhsT=selR[0:1, :], rhs=rs,
                                     start=True, stop=True)
                    bcs2 = pB.tile([64, 512], bf16, name="bcs2", tag="bc2",
                                   bufs=2)
                    nc.scalar.activation(out=bcs2, in_=bcn2[0:64], func=AF.Copy)
                    r2 = 64 * (h % 2)
                    nc.vector.tensor_mul(OT_n[r2:r2 + 64, h // 2, jsl],
                                         O[0:64], bcs2)
                # proj partials for this token half (psum shared with psS)
                for ob in range(8):
                    ps = psS.tile([128, 512], f32, name="pps", tag="s")
                    for rc in range(2):
                        nc.tensor.matmul(
                            ps,
                            lhsT=wproj_sb[:, rc, ob * 128:(ob + 1) * 128],
                            rhs=OT_n[:, rc, jsl],
                            start=(rc == 0), stop=(rc == 1))
                    if ob % 2 == 0:
                        nc.scalar.activation(out=projT[:, ob, jsl], in_=ps,
                                             func=AF.Identity,
                                             bias=bproj_sb[:, ob:ob + 1],
                                             scale=1.0)
                    else:
                        nc.vector.tensor_scalar(
                            out=projT[:, ob, jsl], in0=ps,
                            scalar1=bproj_sb[:, ob:ob + 1], scalar2=None,
                            op0=OP.add)
                for q in range(4):
                    t0 = 512 * j + 128 * q
                    nc.sync.dma_start(bounce1[j][q],
                                      projT[:, :, t0:t0 + 128])
                nc.gpsimd.collective_compute(
                    "ReduceScatter", OP.add,
                    ins=[bounce1[j].opt()], outs=[rs_d[j][:].opt()],
                    replica_groups=groups)
                nc.sync.dma_start(rs_sb[:, :, j, :], rs_d[j][:])

        # ============ Phase C: residual + LN2 + MLP on own 256 tokens ======
        TQ = 256
        with tc.tile_pool(name="pC", bufs=1) as pC, \
             tc.tile_pool(name="psM", bufs=4, space="PSUM") as psM, \
             tc.tile_pool(name="psLN2", bufs=1, space="PSUM") as psLN2, \
             tc.tile_pool(name="psBC2", bufs=1, space="PSUM") as psBC2:
            x1q = pC.tile([128, 8, 2, 128], f32r, name="x1q")
            y2q = pC.tile([128, 8, TQ], bf16, name="y2q")
            h1q = pC.tile([128, 32, TQ], bf16, name="h1q")
            outq = pC.tile([128, 8, 2, 128], bf16, name="outq")
            # residual: x1 = x(own tokens) + scattered proj sum
            # own tokens of half j: [512j + 128g, +128) -- g-dependent slice
            # via an input-provided iota? g is baked per-core through xq_in.
            for j in range(2):
                for cc in range(8):
                    eng = nc.vector if cc % 2 == 0 else nc.gpsimd
                    eng.tensor_add(x1q[:, cc, j, :], xq_sb[:, cc, j, :],
                                   rs_sb[:, cc, j, :])
            # LN2 over own 256 tokens
            st_mu2 = psLN2.tile([1, TQ], f32, name="st_mu2")
            st_sq2 = psLN2.tile([1, TQ], f32, name="st_sq2")
            for cc in range(8):
                nc.tensor.matmul(st_mu2, lhsT=ones128,
                                 rhs=x1q[:, cc].rearrange("p a b -> p (a b)"),
                                 start=(cc == 0), stop=(cc == 7),
                                 skip_group_check=True)
            for cc in range(8):
                xq2 = ptmpA.tile([128, TQ], f32r, name="xq2", tag="xq2", bufs=2)
                nc.scalar.activation(
                    out=xq2, in_=x1q[:, cc].rearrange("p a b -> p (a b)"),
                    func=AF.Square)
                nc.tensor.matmul(st_sq2, lhsT=ones128, rhs=xq2,
                                 start=(cc == 0), stop=(cc == 7),
                                 skip_group_check=True)
            tmu = ptmpA.tile([1, TQ], f32, name="tmu2", tag="tmu2", bufs=1)
            tms = ptmpA.tile([1, TQ], f32, name="tms2", tag="tms2", bufs=1)
            tvv = ptmpA.tile([1, TQ], f32, name="tvv2", tag="tvv2", bufs=1)
            trs = ptmpA.tile([1, TQ], f32r, name="trs2", tag="trs2", bufs=1)
            tmr = ptmpA.tile([1, TQ], f32r, name="tmr2", tag="tmr2", bufs=1)
            nc.vector.tensor_scalar(out=tmu, in0=st_mu2, scalar1=1.0 / C,
                                    scalar2=None, op0=OP.mult)
            nc.scalar.activation(out=tms, in_=tmu, func=AF.Square)
            nc.vector.tensor_scalar(out=tvv, in0=st_sq2, scalar1=1.0 / C,
                                    scalar2=None, op0=OP.mult)
            nc.vector.tensor_sub(tvv, tvv, tms)
            nc.scalar.activation(out=tms, in_=tvv, func=AF.Sqrt,
                                 bias=eps_sb[0:1, 0:1])
            nc.vector.reciprocal(trs, tms)
            nc.vector.tensor_mul(tmr, trs, tmu)
            bc2 = psBC2.tile([128, 2, TQ], f32, name="bc2")
            for s, row in enumerate((trs, tmr)):
                nc.tensor.matmul(bc2[:, s, :], lhsT=selR[0:1, :], rhs=row,
                                 start=True, stop=True)
            bcs = ptmpA.tile([128, 2, TQ], f32, name="bcs2c", tag="bcs2c",
                             bufs=1)
            nc.vector.tensor_copy(bcs, bc2)
            for cc in range(8):
                eng = nc.vector if cc % 2 == 0 else nc.gpsimd
                tp = (ptmpV if cc % 2 == 0 else ptmpG).tile(
                    [128, TQ], f32, name="aplq", tag="aplq", bufs=2)
                eng.tensor_mul(tp, x1q[:, cc].rearrange("p a b -> p (a b)"),
                               bcs[:, 0, :])
                eng.tensor_sub(y2q[:, cc, :], tp, bcs[:, 1, :])
            # fc1 + gelu (full hidden, streamed weights)
            for hb in range(32):
                w1t = pw1.tile([128, 8, 128], bf16, name="w1t", tag="w1")
                nc.sync.dma_start(w1t, w1_in[hb])
                ps = psM.tile([128, TQ], f32, name="m1ps", tag="mm")
                for cc in range(8):
                    nc.tensor.matmul(ps, lhsT=w1t[:, cc, :],
                                     rhs=y2q[:, cc, :],
                                     start=(cc == 0), stop=(cc == 7))
                nc.scalar.activation(out=h1q[:, hb, :], in_=ps, func=AF.Gelu,
                                     bias=b1_sb[:, hb:hb + 1], scale=1.0)
            # fc2 (full hidden contraction -> complete out for own tokens)
            for ob in range(8):
                w2t = pw2.tile([128, 32, 128], bf16, name="w2t", tag="w2")
                nc.sync.dma_start(w2t, w2_in[ob])
                ps = psM.tile([128, TQ], f32, name="m2ps", tag="mm")
                for hc in range(32):
                    nc.tensor.matmul(ps, lhsT=w2t[:, hc, :],
                                     rhs=h1q[:, hc, :],
                                     start=(hc == 0), stop=(hc == 31))
                nc.scalar.activation(out=outq[:, ob, :].rearrange(
                    "p a b -> p (a b)"), in_=ps, func=AF.Identity,
                    bias=b2_sb[:, ob:ob + 1], scale=1.0)
            for j in range(2):
                for cc in range(8):
                    eng = nc.vector if cc % 2 == 0 else nc.gpsimd
                    eng.tensor_add(outq[:, cc, j, :], outq[:, cc, j, :],
                                   x1q[:, cc, j, :])
            for cc in range(8):
                nc.sync.dma_start(out_ext[cc], outq[:, cc].rearrange(
                    "p a b -> p (a b)"))

    nc.finalize()
    return nc


def _get_nc():
    if "nc" not in _CACHE:
        _CACHE["nc"] = _build_nc()
    return _CACHE["nc"]


def _make_in_maps(inputs):
    import ml_dtypes
    bf = ml_dtypes.bfloat16
    x = np.asarray(inputs["x"], np.float32)
    mask = np.asarray(inputs["padding_mask"]).astype(bool)
    alibi = np.asarray(inputs["alibi_bias"], np.float32)
    g1 = np.asarray(inputs["ln1_g"], np.float32)
    b1ln = np.asarray(inputs["ln1_b"], np.float32)
    g2 = np.asarray(inputs["ln2_g"], np.float32)
    b2ln = np.asarray(inputs["ln2_b"], np.float32)
    Wqkv = np.asarray(inputs["Wqkv"], np.float32)
    bqkv = np.asarray(inputs["bqkv"], np.float32)
    Wproj = np.asarray(inputs["Wproj"], np.float32)
    bproj = np.asarray(inputs["bproj"], np.float32)
    W1 = np.asarray(inputs["W1"], np.float32)
    b1 = np.asarray(inputs["b1"], np.float32)
    W2 = np.asarray(inputs["W2"], np.float32)
    b2 = np.asarray(inputs["b2"], np.float32)
    ls = np.asarray(inputs["logit_scale"], np.float32).reshape(H)
    scale = np.exp(np.minimum(ls, math.log(100.0))).astype(np.float32)

    Wqkv_eff = g1[:, None] * Wqkv
    bqkv_eff = b1ln @ Wqkv + bqkv
    W1_eff = g2[:, None] * W1
    b1_eff = b2ln @ W1 + b1

    cf = np.zeros((128, 900), np.float32)
    cf[:, 0] = 1.0
    cf[0, 1:65] = 1.0       # sel2T row0 -> p<64
    cf[1, 65:129] = 1.0     # sel2T row1 -> p>=64
    cf[0, 129:257] = 1.0    # selR
    cf[1, 257:385] = 1.0    # selM
    for f in range(4):
        c0 = 385 + 128 * f
        cf[2 * f, c0:c0 + 64] = 1.0
        cf[2 * f + 1, c0 + 64:c0 + 128] = 1.0
    cbm = np.zeros((128, 160), np.float32)
    cbm[:, 0:128] = np.eye(128, dtype=np.float32)
    for f in range(4):
        cbm[0:64, 128 + 8 * f + 2 * f] = 1.0
        cbm[64:128, 128 + 8 * f + 2 * f + 1] = 1.0

    maskadd = np.where(mask, np.float32(-1e9), np.float32(0.0))

    # full MLP weights, tiled for streaming (same for every core)
    w1q = np.ascontiguousarray(
        W1_eff.reshape(8, 128, 32, 128).transpose(2, 1, 0, 3)).astype(bf)
    w2q = np.ascontiguousarray(
        W2.reshape(32, 128, 8, 128).transpose(2, 1, 0, 3)).astype(bf)
    b1T = np.ascontiguousarray(b1_eff.reshape(32, 128).T)
    b2T = np.ascontiguousarray(b2.reshape(8, 128).T)

    in_maps = []
    for c in range(NCORES):
        b, g = divmod(c, GROUP)
        heads = [4 * g + jj for jj in range(4)]
        # qkv column gather: fb order q01 q23 k01 k23 v01 v23
        cols = []
        for f in range(6):
            t = [0, 0, 1, 1, 2, 2][f]
            pair = (heads[0], heads[1]) if f % 2 == 0 else (heads[2], heads[3])
            for hh in pair:
                cols.extend(range(t * C + hh * D, t * C + hh * D + D))
        cols = np.array(cols)
        rows_proj = np.concatenate(
            [np.arange(hh * D, (hh + 1) * D) for hh in heads])

        a = alibi[b, heads].transpose(0, 2, 1) + maskadd[b][None, :, None]
        eal = np.exp(a).astype(bf).reshape(4, 8, 128, N)

        xTb = np.ascontiguousarray(x[b].T).reshape(8, 128, N)
        # own tokens: half j -> [512j + 128g, +128)
        xq = np.stack([xTb[:, :, 512 * j + 128 * g:512 * j + 128 * g + 128]
                       for j in range(2)], axis=2)  # [8,128,2,128]
        m = {
            "xT": xTb,
            "xq": np.ascontiguousarray(xq),
            "ealibi": np.ascontiguousarray(eal),
            "wqkv": np.ascontiguousarray(
                Wqkv_eff[:, cols].reshape(8, 128, 768)).astype(bf),
            "wproj": np.ascontiguousarray(
                Wproj[rows_proj, :].reshape(2, 128, 1024)).astype(bf),
            "w1": w1q,
            "w2": w2q,
            "bqkvT": np.ascontiguousarray(bqkv_eff[cols].reshape(6, 128).T),
            "bprojT": np.ascontiguousarray(bproj.reshape(8, 128).T)
            if g == 0 else np.zeros((128, 8), np.float32),
            "b1T": b1T,
            "b2T": b2T,
            "scalesq": np.ascontiguousarray(scale[heads].reshape(4, 1)),
            "nbT": np.ascontiguousarray(
                np.tile(-scale[heads][None, :], (128, 1))),
            "cf": cf,
            "cb": cbm.astype(bf),
        }
        in_maps.append(m)
    return in_maps


def _run(inputs, trace=False):
    from concourse import bass_utils
    nc = _get_nc()
    in_maps = _make_in_maps(inputs)
    res = bass_utils.run_bass_kernel_spmd(
        nc, in_maps, core_ids=list(range(NCORES)), trace=trace)
    y = np.zeros((B, N, C), np.float32)
    for c in range(NCORES):
        b, g = divmod(c, GROUP)
        o = np.asarray(res.results[c]["out"]).astype(np.float32)
        o = o.reshape(C, 2, 128)              # [cc*128+p, j, t]
        for j in range(2):
            t0 = 512 * j + 128 * g
            y[b, t0:t0 + 128, :] = o[:, j, :].T
    return y, res


def kernel(**inputs):
    y, _ = _run(inputs, trace=False)
    return y
